# revision 6
# baseline (speedup 1.0000x reference)
"""Binary Jaccard index (IoU) kernel for Trainium2, 8 NeuronCores.

Reference computation (B=32, C=3, H=512, W=512, f32):
    a = (input >= 0.5), b = (target >= 0.5)
    inter[b,c] = sum_hw(a*b); union = sum(a) + sum(b) - inter
    iou = inter/union (1.0 where union == 0); return mean(iou)

Strategy: pure data parallel over the batch dim -- each of the 8 cores gets
4 batches = 12 (b,c) pairs, each pair a [128, 2048] f32 plane. Per pair,
3 fused DVE ops produce the three per-partition partial sums directly:
  1. tensor_scalar(is_ge 0.5, accum add) : a-plane (bf16) + row-sums of a
  2. tensor_scalar(is_ge 0.5, accum add) : b-plane (bf16) + row-sums of b
  3. scalar_tensor_tensor(bypass, mult)  : a*b plane (bf16) + row-sums of a*b
Row-sums land in columns of a [128, 36] stats tile; one DMA writes it out.
The final partition-sums + IoU + mean over 96 pairs are a trivial host-side
epilogue on 8x[128,36] floats (sums are integer-valued, exact in f32).
"""

import numpy as np

import concourse.bacc as bacc
import concourse.bass as bass
import concourse.mybir as mybir
import concourse.tile as tile
from concourse.bass_utils import run_bass_kernel_spmd

N_CORES = 8
B, C, H, W = 32, 3, 512, 512
B_LOCAL = B // N_CORES          # 4 batches per core
PAIRS = B_LOCAL * C             # 12 (batch, channel) pairs per core
P = 128                         # SBUF partitions
F = (H * W) // P                # 2048 free-dim elements per pair
THRESHOLD = 0.5

_CACHE = {}


def build_nc() -> bass.Bass:
    nc = bacc.Bacc("TRN2", target_bir_lowering=False, debug=False,
                   num_devices=N_CORES)
    x_d = nc.dram_tensor("x", [PAIRS, P, F], mybir.dt.float32,
                         kind="ExternalInput").ap()
    t_d = nc.dram_tensor("t", [PAIRS, P, F], mybir.dt.float32,
                         kind="ExternalInput").ap()
    s_d = nc.dram_tensor("stats", [P, PAIRS * 3], mybir.dt.float32,
                         kind="ExternalOutput").ap()

    with tile.TileContext(nc) as tc:
        with tc.tile_pool(name="io", bufs=3) as io_pool, \
             tc.tile_pool(name="planes", bufs=2) as plane_pool, \
             tc.tile_pool(name="acc", bufs=1) as acc_pool:
            stats = acc_pool.tile([P, PAIRS * 3], mybir.dt.float32)
            for i in range(PAIRS):
                xt = io_pool.tile([P, F], mybir.dt.float32, tag="x")
                tt = io_pool.tile([P, F], mybir.dt.float32, tag="t")
                nc.sync.dma_start(out=xt, in_=x_d[i])
                nc.sync.dma_start(out=tt, in_=t_d[i])
                a = plane_pool.tile([P, F], mybir.dt.bfloat16, tag="a")
                b = plane_pool.tile([P, F], mybir.dt.bfloat16, tag="b")
                ab = plane_pool.tile([P, F], mybir.dt.bfloat16, tag="ab")
                nc.vector.tensor_scalar(
                    out=a, in0=xt, scalar1=THRESHOLD, scalar2=None,
                    op0=mybir.AluOpType.is_ge, op1=mybir.AluOpType.add,
                    accum_out=stats[:, 3 * i:3 * i + 1])
                nc.vector.tensor_scalar(
                    out=b, in0=tt, scalar1=THRESHOLD, scalar2=None,
                    op0=mybir.AluOpType.is_ge, op1=mybir.AluOpType.add,
                    accum_out=stats[:, 3 * i + 1:3 * i + 2])
                nc.vector.scalar_tensor_tensor(
                    out=ab, in0=a, scalar=1.0, in1=b,
                    op0=mybir.AluOpType.bypass, op1=mybir.AluOpType.mult,
                    accum_out=stats[:, 3 * i + 2:3 * i + 3])
            nc.sync.dma_start(out=s_d, in_=stats)
    nc.compile()
    return nc


def shard_inputs(input: np.ndarray, target: np.ndarray) -> list[dict]:
    in_maps = []
    for c in range(N_CORES):
        xs = input[c * B_LOCAL:(c + 1) * B_LOCAL].reshape(PAIRS, P, F)
        ts = target[c * B_LOCAL:(c + 1) * B_LOCAL].reshape(PAIRS, P, F)
        in_maps.append({"x": np.ascontiguousarray(xs),
                        "t": np.ascontiguousarray(ts)})
    return in_maps


def combine_outputs(stats_per_core: list[np.ndarray]) -> np.float32:
    ious = []
    for s in stats_per_core:
        sums = s.astype(np.float64).sum(axis=0).reshape(PAIRS, 3)
        sa, sb, sab = sums[:, 0], sums[:, 1], sums[:, 2]
        inter = sab
        union = sa + sb - sab
        iou = np.where(union > 0, inter / np.where(union > 0, union, 1.0), 1.0)
        ious.append(iou.astype(np.float32))
    return np.float32(np.mean(np.concatenate(ious)))


def kernel(input: np.ndarray, target: np.ndarray) -> np.ndarray:
    input = np.asarray(input, dtype=np.float32)
    target = np.asarray(target, dtype=np.float32)
    assert input.shape == (B, C, H, W) and target.shape == (B, C, H, W)

    if "nc" not in _CACHE:
        _CACHE["nc"] = build_nc()
    nc = _CACHE["nc"]

    res = run_bass_kernel_spmd(nc, shard_inputs(input, target),
                               core_ids=list(range(N_CORES)))
    return combine_outputs([r["stats"] for r in res.results])


# revision 9
# speedup vs baseline: 1.0302x; 1.0302x over previous
"""Binary Jaccard index (IoU) kernel for Trainium2, 8 NeuronCores.

Reference computation (B=32, C=3, H=512, W=512, f32):
    a = (input >= 0.5), b = (target >= 0.5)
    inter[b,c] = sum_hw(a*b); union = sum(a) + sum(b) - inter
    iou = inter/union (1.0 where union == 0); return mean(iou)

Strategy: pure data parallel over the batch dim -- each of the 8 cores gets
4 batches = 12 (b,c) pairs, each pair a [128, 2048] f32 plane. Per pair,
3 fused DVE ops produce the three per-partition partial sums directly:
  1. tensor_scalar(is_ge 0.5, accum add) : a-plane (bf16) + row-sums of a
  2. tensor_scalar(is_ge 0.5, accum add) : b-plane (bf16) + row-sums of b
  3. scalar_tensor_tensor(bypass, mult)  : a*b plane (bf16) + row-sums of a*b
Row-sums land in columns of a [128, 36] stats tile; one DMA writes it out.
The final partition-sums + IoU + mean over 96 pairs are a trivial host-side
epilogue on 8x[128,36] floats (sums are integer-valued, exact in f32).
"""

import numpy as np

import concourse.bacc as bacc
import concourse.bass as bass
import concourse.mybir as mybir
import concourse.tile as tile
from concourse.bass_utils import run_bass_kernel_spmd

N_CORES = 8
B, C, H, W = 32, 3, 512, 512
B_LOCAL = B // N_CORES          # 4 batches per core
PAIRS = B_LOCAL * C             # 12 (batch, channel) pairs per core
P = 128                         # SBUF partitions
F = (H * W) // P                # 2048 free-dim elements per pair
CHUNKS = 4                      # split each pair into chunks for finer overlap
FC = F // CHUNKS
THRESHOLD = 0.5

_CACHE = {}


def build_nc() -> bass.Bass:
    nc = bacc.Bacc("TRN2", target_bir_lowering=False, debug=False,
                   num_devices=N_CORES)
    x_d = nc.dram_tensor("x", [PAIRS, P, F], mybir.dt.float32,
                         kind="ExternalInput").ap()
    t_d = nc.dram_tensor("t", [PAIRS, P, F], mybir.dt.float32,
                         kind="ExternalInput").ap()
    s_d = nc.dram_tensor("stats", [P, PAIRS * CHUNKS * 3], mybir.dt.float32,
                         kind="ExternalOutput").ap()

    with tile.TileContext(nc) as tc:
        with tc.tile_pool(name="io", bufs=4) as io_pool, \
             tc.tile_pool(name="planes", bufs=2) as plane_pool, \
             tc.tile_pool(name="acc", bufs=1) as acc_pool:
            stats = acc_pool.tile([P, PAIRS * CHUNKS * 3], mybir.dt.float32)
            col = 0
            for i in range(PAIRS):
                for c in range(CHUNKS):
                    xt = io_pool.tile([P, FC], mybir.dt.float32, tag="x")
                    tt = io_pool.tile([P, FC], mybir.dt.float32, tag="t")
                    nc.sync.dma_start(out=xt, in_=x_d[i, :, c * FC:(c + 1) * FC])
                    nc.sync.dma_start(out=tt, in_=t_d[i, :, c * FC:(c + 1) * FC])
                    a = plane_pool.tile([P, FC], mybir.dt.bfloat16, tag="a")
                    b = plane_pool.tile([P, FC], mybir.dt.bfloat16, tag="b")
                    ab = plane_pool.tile([P, FC], mybir.dt.bfloat16, tag="ab")
                    nc.vector.tensor_scalar(
                        out=a, in0=xt, scalar1=THRESHOLD, scalar2=None,
                        op0=mybir.AluOpType.is_ge, op1=mybir.AluOpType.add,
                        accum_out=stats[:, col:col + 1])
                    nc.vector.tensor_scalar(
                        out=b, in0=tt, scalar1=THRESHOLD, scalar2=None,
                        op0=mybir.AluOpType.is_ge, op1=mybir.AluOpType.add,
                        accum_out=stats[:, col + 1:col + 2])
                    nc.vector.scalar_tensor_tensor(
                        out=ab, in0=a, scalar=1.0, in1=b,
                        op0=mybir.AluOpType.bypass, op1=mybir.AluOpType.mult,
                        accum_out=stats[:, col + 2:col + 3])
                    col += 3
            nc.sync.dma_start(out=s_d, in_=stats)
    nc.compile()
    return nc


def shard_inputs(input: np.ndarray, target: np.ndarray) -> list[dict]:
    in_maps = []
    for c in range(N_CORES):
        xs = input[c * B_LOCAL:(c + 1) * B_LOCAL].reshape(PAIRS, P, F)
        ts = target[c * B_LOCAL:(c + 1) * B_LOCAL].reshape(PAIRS, P, F)
        in_maps.append({"x": np.ascontiguousarray(xs),
                        "t": np.ascontiguousarray(ts)})
    return in_maps


def combine_outputs(stats_per_core: list[np.ndarray]) -> np.float32:
    ious = []
    for s in stats_per_core:
        # columns: [pair, chunk, quantity]; sum over partitions and chunks
        sums = s.astype(np.float64).sum(axis=0).reshape(PAIRS, CHUNKS, 3).sum(axis=1)
        sa, sb, sab = sums[:, 0], sums[:, 1], sums[:, 2]
        inter = sab
        union = sa + sb - sab
        iou = np.where(union > 0, inter / np.where(union > 0, union, 1.0), 1.0)
        ious.append(iou.astype(np.float32))
    return np.float32(np.mean(np.concatenate(ious)))


def kernel(input: np.ndarray, target: np.ndarray) -> np.ndarray:
    input = np.asarray(input, dtype=np.float32)
    target = np.asarray(target, dtype=np.float32)
    assert input.shape == (B, C, H, W) and target.shape == (B, C, H, W)

    if "nc" not in _CACHE:
        _CACHE["nc"] = build_nc()
    nc = _CACHE["nc"]

    res = run_bass_kernel_spmd(nc, shard_inputs(input, target),
                               core_ids=list(range(N_CORES)))
    return combine_outputs([r["stats"] for r in res.results])
